# revision 27
# baseline (speedup 1.0000x reference)
"""Trainium2 Bass kernel for a 32-head causal attention layer.

Problem: B=1, S=2048, D=4096, 32 heads x 128 head-dim, fp32, llama-style
interleaved RoPE on q/k, KV-cache index_copy (identity for arange indexes),
additive mask + softmax, output projection.

Sharding (8 NeuronCores, tensor-parallel by heads):
  - core c owns heads [4c, 4c+4): wq/wk/wv output rows [512c, 512c+512)
  - per-core: QKV projections -> RoPE -> attention -> attn.T shard [512, 2048]
  - 4 chunked on-device AllGathers (one per 512-query block)
  - core c computes output column shard out[:, 512c:512c+512] = attn @ wo_c.T
  - host concatenates the 8 column shards (pure unshard, no arithmetic)

Pipelined schedule: causal attention for query block qb only needs K/V of
seq chunks <= qb, so the program interleaves
  QKV(0), attn(0), AG(0), QKV(1), attn(1), AG(1), P4(0), QKV(2), ...
which starts the collectives ~4x earlier, overlaps the output projection
with later QKV/attention compute, and keeps the tensor engine continuously
busy (holding its p-state at max clock).

All matmul operands are bf16 (1 cycle/row on the PE, half the DMA bytes of
fp32r); PSUM accumulation stays fp32.  End-to-end max-abs error vs the fp32
reference is ~3e-3 of the output max (budget 2e-2).

RoPE trick: weight rows of wq/wk are permuted per head on the host so the
interleaved pairs (2j, 2j+1) become (j, j+64).  Scores are invariant under
a per-head orthogonal permutation applied to both q and k, and the rotation
then only needs partition-range [0:64]/[64:128] cross-multiplies, which map
to plain DVE tensor_tensor ops (no strided partition access).

The 1/sqrt(128) score scale is folded into the Exp activation's scale
operand.  Softmax runs over the partition (key) axis: scores are computed
transposed st[k, q] = K Q^T, summed with a ones-vector matmul, and
normalized after the PV matmul via a reciprocal + outer-product broadcast.
"""

import numpy as np

import concourse.bass as bass
import concourse.mybir as mybir
import concourse.tile as tile
from concourse import bacc
from concourse.bass_utils import run_bass_kernel_spmd

F32 = mybir.dt.float32
BF16 = mybir.dt.bfloat16

S = 2048
D = 4096
HD = 128
N_HEADS = 32
N_CORES = 8
HPC = N_HEADS // N_CORES          # heads per core = 4
FC = HPC * HD                     # features per core = 512
N_DC = D // 128                   # 32 contraction chunks
N_SC = S // 512                   # 4 seq chunks of 512
N_KC = S // 128                   # 16 key chunks of 128
SCALE = 1.0 / np.sqrt(HD)
NEG = -1e9


def _build_module(causal: bool):
    nc = bacc.Bacc(num_devices=N_CORES)

    xT = nc.dram_tensor("xT", [N_SC * D, 512], BF16, kind="ExternalInput")
    wqk_t = nc.dram_tensor("wqk_t", [D, 2 * FC], BF16, kind="ExternalInput")
    wv_t = nc.dram_tensor("wv_t", [D, FC], BF16, kind="ExternalInput")
    wo_t = nc.dram_tensor("wo_t", [D, FC], BF16, kind="ExternalInput")
    cosb = nc.dram_tensor("cosb", [128, S], F32, kind="ExternalInput")
    sinb = nc.dram_tensor("sinb", [128, S], F32, kind="ExternalInput")
    if causal:
        bmask = nc.dram_tensor("bmask", [128, 4 * 512], BF16, kind="ExternalInput")
    else:
        maskT = nc.dram_tensor("maskT", [S, S], F32, kind="ExternalInput")
    out_t = nc.dram_tensor("out", [S, FC], F32, kind="ExternalOutput")

    with tile.TileContext(nc) as tc:
        with tc.tile_pool(name="const", bufs=1) as constp, \
             tc.tile_pool(name="dram", bufs=1, space="DRAM") as dram, \
             tc.tile_pool(name="sb", bufs=1) as sb, \
             tc.tile_pool(name="ps", bufs=1, space="PSUM") as ps:
            cc_in = [dram.tile([FC, 512], BF16, name=f"cc_in{i}")
                     for i in range(N_SC)]
            cc_out = [dram.tile([D, 512], BF16, addr_space="Shared",
                                name=f"cc_out{i}") for i in range(N_SC)]
            # block 3's AllGather is split into head-pair halves so the first
            # half is in flight while heads 2-3 are still computing
            cc_outh = [dram.tile([N_CORES * 256, 512], BF16,
                                 addr_space="Shared", name=f"cc_outh{j}")
                       for j in range(2)]

            ones_f = constp.tile([128, 1], F32, tag="ones_f")
            nc.vector.memset(ones_f[:], 1.0)
            ones_col = constp.tile([128, 1], BF16, tag="ones_col")
            nc.vector.tensor_copy(ones_col[:], ones_f[:])

            cos_sb = constp.tile([128, S], F32, tag="cos")
            sin_sb = constp.tile([128, S], F32, tag="sin")
            nc.gpsimd.dma_start(cos_sb[:], cosb[:])
            nc.gpsimd.dma_start(sin_sb[:], sinb[:])
            if causal:
                bm_sb = constp.tile([128, 4 * 512], BF16, tag="bm")
                nc.gpsimd.dma_start(bm_sb[:], bmask[:])

            # persistent q/k/v activation tiles (bf16)
            qt = [constp.tile([128, S], BF16, tag=f"qt{h}", name=f"qt{h}")
                  for h in range(HPC)]
            kt = [constp.tile([128, S], BF16, tag=f"kt{h}", name=f"kt{h}")
                  for h in range(HPC)]
            vt = [constp.tile([128, FC], BF16, tag=f"vt{b}", name=f"vt{b}")
                  for b in range(N_KC)]

            # resident wo tiles: on the gpsimd queue so they do not delay the
            # QKV weight/activation streams (sync+scalar queues)
            wo_sb = [constp.tile([128, FC], BF16, tag=f"wo{dc}", name=f"wo{dc}")
                     for dc in range(N_DC)]
            for dc in range(N_DC):
                nc.gpsimd.dma_start(wo_sb[dc][:], wo_t[dc * 128:(dc + 1) * 128, :])

            def rope_evict(acc, dst, sc):
                s0 = sc * 512
                cs = cos_sb[:, s0:s0 + 512]
                sn = sin_sb[:, s0:s0 + 512]
                t14 = sb.tile([128, 512], F32, tag="t1", bufs=2)
                t2 = sb.tile([128, 512], F32, tag="t2", bufs=2)
                t3 = sb.tile([128, 512], F32, tag="t3", bufs=2)
                mul = mybir.AluOpType.mult
                # acc reads first (3 ops, first full-width: cos_sb holds the
                # cos table duplicated in both halves) so the PSUM bank frees
                # ASAP for the next sweep's matmuls
                nc.vector.tensor_tensor(out=t14[:], in0=acc[:], in1=cs[:],
                                        op=mul)
                nc.vector.tensor_tensor(out=t2[0:64, :], in0=acc[64:128, :],
                                        in1=sn[0:64, :], op=mul)
                nc.vector.tensor_tensor(out=t3[64:128, :], in0=acc[0:64, :],
                                        in1=sn[64:128, :], op=mul)
                nc.vector.tensor_tensor(out=dst[0:64, s0:s0 + 512],
                                        in0=t14[0:64, :], in1=t2[0:64, :],
                                        op=mybir.AluOpType.subtract)
                nc.vector.tensor_tensor(out=dst[64:128, s0:s0 + 512],
                                        in0=t14[64:128, :], in1=t3[64:128, :],
                                        op=mybir.AluOpType.add)

            def qkv_phase(sc):
                s0 = sc * 512
                # q sweep (4 PSUM accumulators), then k sweep, then v sweep.
                xts = []
                for which in ("q", "k"):
                    off = 0 if which == "q" else FC
                    accs = [ps.tile([128, 512], F32, tag="acc", bufs=4,
                                    name=f"{which}ps{sc}_{h}") for h in range(HPC)]
                    for dc in range(N_DC):
                        d0 = dc * 128
                        if which == "q":
                            xt_sb = sb.tile([128, 512], BF16, tag="xt", bufs=40,
                                            name=f"xt{sc}_{dc}")
                            nc.sync.dma_start(
                                xt_sb[:], xT[sc * D + d0:sc * D + d0 + 128, :])
                            xts.append(xt_sb)
                        w_sb = sb.tile([128, FC], BF16, tag="wqk", bufs=12,
                                       name=f"w{which}{sc}_{dc}")
                        weng = nc.scalar if dc % 2 == 0 else nc.sync
                        weng.dma_start(w_sb[:], wqk_t[d0:d0 + 128, off:off + FC])
                        for h in range(HPC):
                            f0 = h * 128
                            nc.tensor.matmul(accs[h][:], w_sb[:, f0:f0 + 128],
                                             xts[dc][:], start=(dc == 0),
                                             stop=(dc == N_DC - 1))
                    dst = qt if which == "q" else kt
                    for h in range(HPC):
                        rope_evict(accs[h], dst[h], sc)

                v_ps = [ps.tile([128, 512], F32, tag="acc", bufs=4,
                                name=f"vps{sc}_{i}") for i in range(4)]
                for dc in range(N_DC):
                    d0 = dc * 128
                    wv_sb = sb.tile([128, FC], BF16, tag="wv", bufs=12,
                                    name=f"wv{sc}_{dc}")
                    veng = nc.scalar if dc % 2 == 0 else nc.sync
                    veng.dma_start(wv_sb[:], wv_t[d0:d0 + 128, :])
                    for sbk in range(4):
                        nc.tensor.matmul(v_ps[sbk][:],
                                         xts[dc][:, sbk * 128:(sbk + 1) * 128],
                                         wv_sb[:], start=(dc == 0),
                                         stop=(dc == N_DC - 1))
                for sbk in range(4):
                    nc.vector.tensor_copy(vt[sc * 4 + sbk][:], v_ps[sbk][:])

            def attn_partial(qb, klo, khi, split_ag):
                """Attention for query block qb over key chunks [klo, khi],
                normalizing and AllGathering the result.  split_ag fires a
                half AllGather after heads 1 and 3 instead of one at the
                end."""
                q0 = qb * 512
                for h in range(HPC):
                    out_ps = ps.tile([128, 512], F32, tag="out", bufs=1,
                                     name=f"o{qb}_{h}_{klo}")
                    sums_ps = ps.tile([1, 512], F32, tag="sums", bufs=1,
                                      name=f"s{qb}_{h}_{klo}")

                    def issue_st(kc):
                        k0 = kc * 128
                        st_ps = ps.tile([128, 512], F32, tag="st", bufs=2,
                                        name=f"st{qb}_{h}_{kc}")
                        nc.tensor.matmul(st_ps[:], kt[h][:, k0:k0 + 128],
                                         qt[h][:, q0:q0 + 512],
                                         start=True, stop=True)
                        e_sb = sb.tile([128, 512], BF16, tag="e", bufs=4,
                                       name=f"e{qb}_{h}_{kc}")
                        if causal:
                            nc.scalar.activation(
                                e_sb[:], st_ps[:],
                                mybir.ActivationFunctionType.Exp,
                                scale=float(SCALE))
                            j = kc - 4 * qb
                            if 0 <= j <= 3:
                                nc.vector.tensor_tensor(
                                    out=e_sb[:], in0=e_sb[:],
                                    in1=bm_sb[:, j * 512:(j + 1) * 512],
                                    op=mybir.AluOpType.mult)
                        else:
                            mt_sb = sb.tile([128, 512], F32, tag="mt", bufs=3,
                                            name=f"mt{qb}_{h}_{kc}")
                            nc.sync.dma_start(
                                mt_sb[:], maskT[k0:k0 + 128, q0:q0 + 512])
                            nc.vector.tensor_tensor(
                                out=st_ps[:], in0=st_ps[:], in1=mt_sb[:],
                                op=mybir.AluOpType.add)
                            nc.scalar.activation(
                                e_sb[:], st_ps[:],
                                mybir.ActivationFunctionType.Exp,
                                scale=float(SCALE))
                        return e_sb

                    e_cur = issue_st(klo)
                    for kc in range(klo, khi + 1):
                        e_next = issue_st(kc + 1) if kc < khi else None
                        nc.tensor.matmul(out_ps[:],
                                         vt[kc][:, h * 128:(h + 1) * 128],
                                         e_cur[:], start=(kc == klo),
                                         stop=(kc == khi))
                        nc.tensor.matmul(sums_ps[:], ones_col[:], e_cur[:],
                                         start=(kc == klo), stop=(kc == khi))
                        e_cur = e_next
                    r_sb = sb.tile([1, 512], F32, tag="r", bufs=2,
                                   name=f"r{qb}_{h}")
                    nc.vector.reciprocal(r_sb[:], sums_ps[:])
                    rb_sb = sb.tile([128, 512], F32, tag="rbs", bufs=2,
                                    name=f"rbs{qb}_{h}")
                    nc.gpsimd.partition_broadcast(rb_sb[:], r_sb[:])
                    a_sb = sb.tile([128, 512], BF16, tag="a", bufs=2,
                                   name=f"a{qb}_{h}")
                    nc.vector.tensor_tensor(out=a_sb[:], in0=out_ps[:],
                                            in1=rb_sb[:],
                                            op=mybir.AluOpType.mult)
                    nc.gpsimd.dma_start(cc_in[qb][h * 128:(h + 1) * 128, :],
                                        a_sb[:])
                    if split_ag and h % 2 == 1:
                        j = h // 2
                        nc.gpsimd.collective_compute(
                            "AllGather",
                            mybir.AluOpType.bypass,
                            replica_groups=[list(range(N_CORES))],
                            ins=[cc_in[qb][j * 256:(j + 1) * 256, :].opt()],
                            outs=[cc_outh[j][:].opt()],
                        )
                if not split_ag:
                    nc.gpsimd.collective_compute(
                        "AllGather",
                        mybir.AluOpType.bypass,
                        replica_groups=[list(range(N_CORES))],
                        ins=[cc_in[qb][:].opt()],
                        outs=[cc_out[qb][:].opt()],
                    )

            def p4_run(sc, chunks, o_ps, i0, n_total):
                # at-loads on the gpsimd queue so a pending AllGather never
                # blocks the QKV weight streams
                for i, (src, r0, dc) in enumerate(chunks):
                    at_sb = sb.tile([128, 512], BF16, tag="at", bufs=8,
                                    name=f"at{sc}_{dc}")
                    nc.gpsimd.dma_start(at_sb[:], src[r0:r0 + 128, :])
                    for sbk in range(4):
                        nc.tensor.matmul(o_ps[sbk][:],
                                         at_sb[:, sbk * 128:(sbk + 1) * 128],
                                         wo_sb[dc][:], start=(i0 + i == 0),
                                         stop=(i0 + i == n_total - 1))

            def p4_evict(sc, o_ps):
                s0 = sc * 512
                for sbk in range(4):
                    o_sb = sb.tile([128, FC], F32, tag="osb", bufs=2,
                                   name=f"osb{sc}_{sbk}")
                    nc.vector.tensor_copy(o_sb[:], o_ps[sbk][:])
                    r0 = s0 + sbk * 128
                    nc.scalar.dma_start(out_t[r0:r0 + 128, :], o_sb[:])

            def p4_phase(sc):
                o_ps = [ps.tile([128, 512], F32, tag="acc", bufs=4,
                                name=f"ops{sc}_{i}") for i in range(4)]
                chunks = [(cc_out[sc], dc * 128, dc) for dc in range(N_DC)]
                p4_run(sc, chunks, o_ps, 0, N_DC)
                p4_evict(sc, o_ps)

            # pipelined schedule; P4(2) is placed after attn(3) so it hides
            # the last AllGather's flight before P4(3) consumes it
            last = N_SC - 1
            kmax_of = (lambda qb: 4 * qb + 3) if causal else (lambda qb: N_KC - 1)
            for sc in range(N_SC):
                qkv_phase(sc)
                attn_partial(sc, 0, kmax_of(sc), split_ag=(sc == last))
                if 1 <= sc < last:
                    p4_phase(sc - 1)
            # tail: P4(3) head-pair 0 first (its half-AllGather landed during
            # attn(3)), then P4(2) on the freed attention PSUM banks to hide
            # the second half-AllGather's flight, then P4(3) head-pair 1
            o3 = [ps.tile([128, 512], F32, tag="acc", bufs=4,
                          name=f"ops{last}_{i}") for i in range(4)]
            ch3 = [(cc_outh[j], c * 256 + hh * 128, c * HPC + 2 * j + hh)
                   for j in range(2) for c in range(N_CORES)
                   for hh in range(2)]
            p4_run(last, ch3[:16], o3, 0, N_DC)
            o2 = [ps.tile([128, 512], F32, tag=t, bufs=b,
                          name=f"ops{last-1}_{i}")
                  for i, (t, b) in enumerate((("st", 2), ("st", 2),
                                              ("out", 1), ("sums", 1)))]
            ch2 = [(cc_out[last - 1], dc * 128, dc) for dc in range(N_DC)]
            p4_run(last - 1, ch2, o2, 0, N_DC)
            p4_evict(last - 1, o2)
            p4_run(last, ch3[16:], o3, 16, N_DC)
            p4_evict(last, o3)

    nc.finalize()
    return nc


_MODULE_CACHE: dict = {}


def _get_module(causal: bool):
    if causal not in _MODULE_CACHE:
        _MODULE_CACHE[causal] = _build_module(causal)
    return _MODULE_CACHE[causal]


def _rope_perm() -> np.ndarray:
    """Per-head permutation: interleaved pairs (2j, 2j+1) -> (j, j+64)."""
    p = np.empty(HD, dtype=np.int64)
    p[0:64] = np.arange(0, HD, 2)
    p[64:128] = np.arange(1, HD, 2)
    full = np.concatenate([h * HD + p for h in range(HPC)])
    return full


def _canonical_causal_mask() -> np.ndarray:
    m = np.where(np.tril(np.ones((S, S), dtype=bool)), np.float32(0.0),
                 np.float32(NEG))
    return m.astype(np.float32)


def _numpy_fallback(x, freqs_cos, freqs_sin, mask, input_indexes, cache_k,
                    cache_v, wq, wk, wv, wo):
    """Exact reference reimplementation (host, fp32). Only used for inputs
    the device kernel does not model (non-arange cache indexes)."""
    B = x.shape[0]
    xf = x.astype(np.float32)

    def rope(t):
        tr = t[..., 0::2]
        ti = t[..., 1::2]
        c = freqs_cos[None, :, None, :]
        s = freqs_sin[None, :, None, :]
        outr = tr * c - ti * s
        outi = tr * s + ti * c
        return np.stack([outr, outi], axis=-1).reshape(t.shape)

    xq = (xf @ wq.T).reshape(B, S, N_HEADS, HD)
    xk = (xf @ wk.T).reshape(B, S, N_HEADS, HD)
    xv = (xf @ wv.T).reshape(B, S, N_HEADS, HD)
    xq = rope(xq)
    xk = rope(xk)
    keys = np.array(cache_k)
    vals = np.array(cache_v)
    keys[:, input_indexes] = xk
    vals[:, input_indexes] = xv
    scores = np.einsum("bqhd,bkhd->bhqk", xq, keys) / np.sqrt(HD)
    scores = scores + mask
    scores = scores - scores.max(axis=-1, keepdims=True)
    e = np.exp(scores)
    probs = e / e.sum(axis=-1, keepdims=True)
    out = np.einsum("bhqk,bkhd->bqhd", probs, vals)
    return (out.reshape(B, S, N_HEADS * HD) @ wo.T).astype(np.float32)


def _bf16(a: np.ndarray):
    import ml_dtypes
    return np.ascontiguousarray(a.astype(ml_dtypes.bfloat16))


def _prepare_in_maps(x, freqs_cos, freqs_sin, mask, wq, wk, wv, wo, causal):
    x2 = np.ascontiguousarray(x.reshape(S, D), dtype=np.float32)
    xTf = x2.T  # [D, S]
    xT = _bf16(np.concatenate(
        [xTf[:, sc * 512:(sc + 1) * 512] for sc in range(N_SC)], axis=0))

    cosb = np.empty((128, S), dtype=np.float32)
    sinb = np.empty((128, S), dtype=np.float32)
    fc = np.asarray(freqs_cos, dtype=np.float32).T  # [64, S]
    fs = np.asarray(freqs_sin, dtype=np.float32).T
    cosb[0:64] = fc
    cosb[64:128] = fc
    sinb[0:64] = fs
    sinb[64:128] = fs

    perm = _rope_perm()

    if causal:
        kl = np.arange(128, dtype=np.int64)[:, None]
        ql = np.arange(512, dtype=np.int64)[None, :]
        bmask = np.concatenate(
            [(kl <= ql - 128 * j).astype(np.float32) for j in range(4)], axis=1)
        bmask = _bf16(bmask)
    else:
        maskT = np.ascontiguousarray(
            (np.asarray(mask, dtype=np.float32)[0, 0].T) / np.float32(SCALE))

    in_maps = []
    for c in range(N_CORES):
        r0 = c * FC
        wq_c = np.asarray(wq[r0:r0 + FC], dtype=np.float32)[perm]
        wk_c = np.asarray(wk[r0:r0 + FC], dtype=np.float32)[perm]
        wqk_c = np.concatenate([wq_c, wk_c], axis=0)      # [1024, D]
        wv_c = np.asarray(wv[r0:r0 + FC], dtype=np.float32)
        wo_c = np.asarray(wo[r0:r0 + FC], dtype=np.float32)
        m = {
            "xT": xT,
            "wqk_t": _bf16(wqk_c.T),
            "wv_t": _bf16(wv_c.T),
            "wo_t": _bf16(wo_c.T),
            "cosb": cosb,
            "sinb": sinb,
        }
        if causal:
            m["bmask"] = bmask
        else:
            m["maskT"] = maskT
        in_maps.append(m)
    return in_maps


def _run(inputs: dict, trace: bool = False):
    x = np.asarray(inputs["x"])
    freqs_cos = np.asarray(inputs["freqs_cos"])
    freqs_sin = np.asarray(inputs["freqs_sin"])
    mask = np.asarray(inputs["mask"], dtype=np.float32)
    input_indexes = np.asarray(inputs["input_indexes"])
    wq = np.asarray(inputs["wq"])
    wk = np.asarray(inputs["wk"])
    wv = np.asarray(inputs["wv"])
    wo = np.asarray(inputs["wo"])

    if not np.array_equal(input_indexes.astype(np.int64), np.arange(S)):
        out = _numpy_fallback(x, freqs_cos, freqs_sin, mask, input_indexes,
                              inputs["cache_k"], inputs["cache_v"], wq, wk, wv, wo)
        return out, None

    causal = np.array_equal(mask[0, 0], _canonical_causal_mask())
    nc = _get_module(causal)
    in_maps = _prepare_in_maps(x, freqs_cos, freqs_sin, mask, wq, wk, wv, wo,
                               causal)
    res = run_bass_kernel_spmd(nc, in_maps, core_ids=list(range(N_CORES)),
                               trace=trace)
    out = np.concatenate([res.results[c]["out"] for c in range(N_CORES)],
                         axis=1)
    return out.reshape(1, S, D).astype(np.float32), res


def kernel(**inputs) -> np.ndarray:
    out, _ = _run(inputs, trace=False)
    return out


# revision 29
# speedup vs baseline: 1.0267x; 1.0267x over previous
"""Trainium2 Bass kernel for a 32-head causal attention layer.

Problem: B=1, S=2048, D=4096, 32 heads x 128 head-dim, fp32, llama-style
interleaved RoPE on q/k, KV-cache index_copy (identity for arange indexes),
additive mask + softmax, output projection.

Sharding (8 NeuronCores, tensor-parallel by heads):
  - core c owns heads [4c, 4c+4): wq/wk/wv output rows [512c, 512c+512)
  - per-core: QKV projections -> RoPE -> attention -> attn.T shard [512, 2048]
  - 4 chunked on-device AllGathers (one per 512-query block)
  - core c computes output column shard out[:, 512c:512c+512] = attn @ wo_c.T
  - host concatenates the 8 column shards (pure unshard, no arithmetic)

Pipelined schedule: causal attention for query block qb only needs K/V of
seq chunks <= qb, so the program interleaves
  QKV(0), attn(0), AG(0), QKV(1), attn(1), AG(1), P4(0), QKV(2), ...
which starts the collectives ~4x earlier, overlaps the output projection
with later QKV/attention compute, and keeps the tensor engine continuously
busy (holding its p-state at max clock).

All matmul operands are bf16 (1 cycle/row on the PE, half the DMA bytes of
fp32r); PSUM accumulation stays fp32.  End-to-end max-abs error vs the fp32
reference is ~3e-3 of the output max (budget 2e-2).

RoPE trick: weight rows of wq/wk are permuted per head on the host so the
interleaved pairs (2j, 2j+1) become (j, j+64).  Scores are invariant under
a per-head orthogonal permutation applied to both q and k, and the rotation
then only needs partition-range [0:64]/[64:128] cross-multiplies, which map
to plain DVE tensor_tensor ops (no strided partition access).

The 1/sqrt(128) score scale is folded into the Exp activation's scale
operand.  Softmax runs over the partition (key) axis: scores are computed
transposed st[k, q] = K Q^T, summed with a ones-vector matmul, and
normalized after the PV matmul via a reciprocal + outer-product broadcast.
"""

import numpy as np

import concourse.bass as bass
import concourse.mybir as mybir
import concourse.tile as tile
from concourse import bacc
from concourse.bass_utils import run_bass_kernel_spmd

F32 = mybir.dt.float32
BF16 = mybir.dt.bfloat16

S = 2048
D = 4096
HD = 128
N_HEADS = 32
N_CORES = 8
HPC = N_HEADS // N_CORES          # heads per core = 4
FC = HPC * HD                     # features per core = 512
N_DC = D // 128                   # 32 contraction chunks
N_SC = S // 512                   # 4 seq chunks of 512
N_KC = S // 128                   # 16 key chunks of 128
SCALE = 1.0 / np.sqrt(HD)
NEG = -1e9


def _build_module(causal: bool):
    nc = bacc.Bacc(num_devices=N_CORES)

    xT = nc.dram_tensor("xT", [N_SC * D, 512], BF16, kind="ExternalInput")
    wqk_t = nc.dram_tensor("wqk_t", [D, 2 * FC], BF16, kind="ExternalInput")
    wv_t = nc.dram_tensor("wv_t", [D, FC], BF16, kind="ExternalInput")
    wo_t = nc.dram_tensor("wo_t", [D, FC], BF16, kind="ExternalInput")
    cosb = nc.dram_tensor("cosb", [128, S], F32, kind="ExternalInput")
    sinb = nc.dram_tensor("sinb", [128, S], F32, kind="ExternalInput")
    if causal:
        bmask = nc.dram_tensor("bmask", [128, 4 * 512], BF16, kind="ExternalInput")
    else:
        maskT = nc.dram_tensor("maskT", [S, S], F32, kind="ExternalInput")
    out_t = nc.dram_tensor("out", [S, FC], F32, kind="ExternalOutput")

    with tile.TileContext(nc) as tc:
        with tc.tile_pool(name="const", bufs=1) as constp, \
             tc.tile_pool(name="dram", bufs=1, space="DRAM") as dram, \
             tc.tile_pool(name="sb", bufs=1) as sb, \
             tc.tile_pool(name="ps", bufs=1, space="PSUM") as ps:
            cc_in = [dram.tile([FC, 512], BF16, name=f"cc_in{i}")
                     for i in range(N_SC)]
            cc_out = [dram.tile([D, 512], BF16, addr_space="Shared",
                                name=f"cc_out{i}") for i in range(N_SC)]
            # block 3's AllGather is split into head-pair halves so the first
            # half is in flight while heads 2-3 are still computing; separate
            # input tiles per half avoid a false WAR between the half-1 heads'
            # writes and the in-flight half-0 collective read
            cc_inh = [dram.tile([256, 512], BF16, name=f"cc_inh{j}")
                      for j in range(2)]
            cc_outh = [dram.tile([N_CORES * 256, 512], BF16,
                                 addr_space="Shared", name=f"cc_outh{j}")
                       for j in range(2)]

            ones_f = constp.tile([128, 1], F32, tag="ones_f")
            nc.vector.memset(ones_f[:], 1.0)
            ones_col = constp.tile([128, 1], BF16, tag="ones_col")
            nc.vector.tensor_copy(ones_col[:], ones_f[:])

            cos_sb = constp.tile([128, S], F32, tag="cos")
            sin_sb = constp.tile([128, S], F32, tag="sin")
            nc.gpsimd.dma_start(cos_sb[:], cosb[:])
            nc.gpsimd.dma_start(sin_sb[:], sinb[:])
            if causal:
                bm_sb = constp.tile([128, 4 * 512], BF16, tag="bm")
                nc.gpsimd.dma_start(bm_sb[:], bmask[:])

            # persistent q/k/v activation tiles (bf16)
            qt = [constp.tile([128, S], BF16, tag=f"qt{h}", name=f"qt{h}")
                  for h in range(HPC)]
            kt = [constp.tile([128, S], BF16, tag=f"kt{h}", name=f"kt{h}")
                  for h in range(HPC)]
            vt = [constp.tile([128, FC], BF16, tag=f"vt{b}", name=f"vt{b}")
                  for b in range(N_KC)]

            # resident wo tiles: on the gpsimd queue so they do not delay the
            # QKV weight/activation streams (sync+scalar queues)
            wo_sb = [constp.tile([128, FC], BF16, tag=f"wo{dc}", name=f"wo{dc}")
                     for dc in range(N_DC)]
            for dc in range(N_DC):
                nc.gpsimd.dma_start(wo_sb[dc][:], wo_t[dc * 128:(dc + 1) * 128, :])

            def rope_evict(acc, dst, sc):
                s0 = sc * 512
                cs = cos_sb[:, s0:s0 + 512]
                sn = sin_sb[:, s0:s0 + 512]
                t14 = sb.tile([128, 512], F32, tag="t1", bufs=2)
                t2 = sb.tile([128, 512], F32, tag="t2", bufs=2)
                t3 = sb.tile([128, 512], F32, tag="t3", bufs=2)
                mul = mybir.AluOpType.mult
                # acc reads first (3 ops, first full-width: cos_sb holds the
                # cos table duplicated in both halves) so the PSUM bank frees
                # ASAP for the next sweep's matmuls
                nc.vector.tensor_tensor(out=t14[:], in0=acc[:], in1=cs[:],
                                        op=mul)
                nc.vector.tensor_tensor(out=t2[0:64, :], in0=acc[64:128, :],
                                        in1=sn[0:64, :], op=mul)
                nc.vector.tensor_tensor(out=t3[64:128, :], in0=acc[0:64, :],
                                        in1=sn[64:128, :], op=mul)
                nc.vector.tensor_tensor(out=dst[0:64, s0:s0 + 512],
                                        in0=t14[0:64, :], in1=t2[0:64, :],
                                        op=mybir.AluOpType.subtract)
                nc.vector.tensor_tensor(out=dst[64:128, s0:s0 + 512],
                                        in0=t14[64:128, :], in1=t3[64:128, :],
                                        op=mybir.AluOpType.add)

            def qkv_phase(sc):
                s0 = sc * 512
                # q sweep (4 PSUM accumulators), then k sweep, then v sweep.
                xts = []
                for which in ("q", "k"):
                    off = 0 if which == "q" else FC
                    accs = [ps.tile([128, 512], F32, tag="acc", bufs=4,
                                    name=f"{which}ps{sc}_{h}") for h in range(HPC)]
                    for dc in range(N_DC):
                        d0 = dc * 128
                        if which == "q":
                            xt_sb = sb.tile([128, 512], BF16, tag="xt", bufs=40,
                                            name=f"xt{sc}_{dc}")
                            nc.sync.dma_start(
                                xt_sb[:], xT[sc * D + d0:sc * D + d0 + 128, :])
                            xts.append(xt_sb)
                        w_sb = sb.tile([128, FC], BF16, tag="wqk", bufs=12,
                                       name=f"w{which}{sc}_{dc}")
                        weng = nc.scalar if dc % 2 == 0 else nc.sync
                        weng.dma_start(w_sb[:], wqk_t[d0:d0 + 128, off:off + FC])
                        for h in range(HPC):
                            f0 = h * 128
                            nc.tensor.matmul(accs[h][:], w_sb[:, f0:f0 + 128],
                                             xts[dc][:], start=(dc == 0),
                                             stop=(dc == N_DC - 1))
                    dst = qt if which == "q" else kt
                    for h in range(HPC):
                        rope_evict(accs[h], dst[h], sc)

                v_ps = [ps.tile([128, 512], F32, tag="acc", bufs=4,
                                name=f"vps{sc}_{i}") for i in range(4)]
                for dc in range(N_DC):
                    d0 = dc * 128
                    wv_sb = sb.tile([128, FC], BF16, tag="wv", bufs=12,
                                    name=f"wv{sc}_{dc}")
                    veng = nc.scalar if dc % 2 == 0 else nc.sync
                    veng.dma_start(wv_sb[:], wv_t[d0:d0 + 128, :])
                    for sbk in range(4):
                        nc.tensor.matmul(v_ps[sbk][:],
                                         xts[dc][:, sbk * 128:(sbk + 1) * 128],
                                         wv_sb[:], start=(dc == 0),
                                         stop=(dc == N_DC - 1))
                for sbk in range(4):
                    nc.vector.tensor_copy(vt[sc * 4 + sbk][:], v_ps[sbk][:])

            def attn_partial(qb, klo, khi, split_ag):
                """Attention for query block qb over key chunks [klo, khi],
                normalizing and AllGathering the result.  split_ag fires a
                half AllGather after heads 1 and 3 instead of one at the
                end."""
                q0 = qb * 512
                for h in range(HPC):
                    out_ps = ps.tile([128, 512], F32, tag="out", bufs=1,
                                     name=f"o{qb}_{h}_{klo}")
                    sums_ps = ps.tile([1, 512], F32, tag="sums", bufs=1,
                                      name=f"s{qb}_{h}_{klo}")

                    def issue_st(kc):
                        k0 = kc * 128
                        st_ps = ps.tile([128, 512], F32, tag="st", bufs=2,
                                        name=f"st{qb}_{h}_{kc}")
                        nc.tensor.matmul(st_ps[:], kt[h][:, k0:k0 + 128],
                                         qt[h][:, q0:q0 + 512],
                                         start=True, stop=True)
                        e_sb = sb.tile([128, 512], BF16, tag="e", bufs=4,
                                       name=f"e{qb}_{h}_{kc}")
                        if causal:
                            nc.scalar.activation(
                                e_sb[:], st_ps[:],
                                mybir.ActivationFunctionType.Exp,
                                scale=float(SCALE))
                            j = kc - 4 * qb
                            if 0 <= j <= 3:
                                nc.vector.tensor_tensor(
                                    out=e_sb[:], in0=e_sb[:],
                                    in1=bm_sb[:, j * 512:(j + 1) * 512],
                                    op=mybir.AluOpType.mult)
                        else:
                            mt_sb = sb.tile([128, 512], F32, tag="mt", bufs=3,
                                            name=f"mt{qb}_{h}_{kc}")
                            nc.sync.dma_start(
                                mt_sb[:], maskT[k0:k0 + 128, q0:q0 + 512])
                            nc.vector.tensor_tensor(
                                out=st_ps[:], in0=st_ps[:], in1=mt_sb[:],
                                op=mybir.AluOpType.add)
                            nc.scalar.activation(
                                e_sb[:], st_ps[:],
                                mybir.ActivationFunctionType.Exp,
                                scale=float(SCALE))
                        return e_sb

                    e_cur = issue_st(klo)
                    for kc in range(klo, khi + 1):
                        e_next = issue_st(kc + 1) if kc < khi else None
                        nc.tensor.matmul(out_ps[:],
                                         vt[kc][:, h * 128:(h + 1) * 128],
                                         e_cur[:], start=(kc == klo),
                                         stop=(kc == khi))
                        nc.tensor.matmul(sums_ps[:], ones_col[:], e_cur[:],
                                         start=(kc == klo), stop=(kc == khi))
                        e_cur = e_next
                    r_sb = sb.tile([1, 512], F32, tag="r", bufs=2,
                                   name=f"r{qb}_{h}")
                    nc.vector.reciprocal(r_sb[:], sums_ps[:])
                    rb_sb = sb.tile([128, 512], F32, tag="rbs", bufs=2,
                                    name=f"rbs{qb}_{h}")
                    nc.gpsimd.partition_broadcast(rb_sb[:], r_sb[:])
                    a_sb = sb.tile([128, 512], BF16, tag="a", bufs=2,
                                   name=f"a{qb}_{h}")
                    nc.vector.tensor_tensor(out=a_sb[:], in0=out_ps[:],
                                            in1=rb_sb[:],
                                            op=mybir.AluOpType.mult)
                    if split_ag:
                        nc.gpsimd.dma_start(
                            cc_inh[h // 2][(h % 2) * 128:(h % 2 + 1) * 128, :],
                            a_sb[:])
                    else:
                        nc.gpsimd.dma_start(
                            cc_in[qb][h * 128:(h + 1) * 128, :], a_sb[:])
                    if split_ag and h % 2 == 1:
                        j = h // 2
                        nc.gpsimd.collective_compute(
                            "AllGather",
                            mybir.AluOpType.bypass,
                            replica_groups=[list(range(N_CORES))],
                            ins=[cc_inh[j][:].opt()],
                            outs=[cc_outh[j][:].opt()],
                        )
                if not split_ag:
                    nc.gpsimd.collective_compute(
                        "AllGather",
                        mybir.AluOpType.bypass,
                        replica_groups=[list(range(N_CORES))],
                        ins=[cc_in[qb][:].opt()],
                        outs=[cc_out[qb][:].opt()],
                    )

            def p4_run(sc, chunks, o_ps, i0, n_total):
                # at-loads on the gpsimd queue so a pending AllGather never
                # blocks the QKV weight streams
                for i, (src, r0, dc) in enumerate(chunks):
                    at_sb = sb.tile([128, 512], BF16, tag="at", bufs=8,
                                    name=f"at{sc}_{dc}")
                    nc.gpsimd.dma_start(at_sb[:], src[r0:r0 + 128, :])
                    for sbk in range(4):
                        nc.tensor.matmul(o_ps[sbk][:],
                                         at_sb[:, sbk * 128:(sbk + 1) * 128],
                                         wo_sb[dc][:], start=(i0 + i == 0),
                                         stop=(i0 + i == n_total - 1))

            def p4_evict(sc, o_ps):
                s0 = sc * 512
                for sbk in range(4):
                    o_sb = sb.tile([128, FC], F32, tag="osb", bufs=2,
                                   name=f"osb{sc}_{sbk}")
                    nc.vector.tensor_copy(o_sb[:], o_ps[sbk][:])
                    r0 = s0 + sbk * 128
                    nc.scalar.dma_start(out_t[r0:r0 + 128, :], o_sb[:])

            def p4_phase(sc):
                o_ps = [ps.tile([128, 512], F32, tag="acc", bufs=4,
                                name=f"ops{sc}_{i}") for i in range(4)]
                chunks = [(cc_out[sc], dc * 128, dc) for dc in range(N_DC)]
                p4_run(sc, chunks, o_ps, 0, N_DC)
                p4_evict(sc, o_ps)

            # pipelined schedule; P4(2) is placed after attn(3) so it hides
            # the last AllGather's flight before P4(3) consumes it
            last = N_SC - 1
            kmax_of = (lambda qb: 4 * qb + 3) if causal else (lambda qb: N_KC - 1)
            for sc in range(N_SC):
                qkv_phase(sc)
                attn_partial(sc, 0, kmax_of(sc), split_ag=(sc == last))
                if 1 <= sc < last:
                    p4_phase(sc - 1)
            # tail: P4(3) head-pair 0 first (its half-AllGather landed during
            # attn(3)), then P4(2) on the freed attention PSUM banks to hide
            # the second half-AllGather's flight, then P4(3) head-pair 1
            o3 = [ps.tile([128, 512], F32, tag="acc", bufs=4,
                          name=f"ops{last}_{i}") for i in range(4)]
            ch3 = [(cc_outh[j], c * 256 + hh * 128, c * HPC + 2 * j + hh)
                   for j in range(2) for c in range(N_CORES)
                   for hh in range(2)]
            p4_run(last, ch3[:16], o3, 0, N_DC)
            o2 = [ps.tile([128, 512], F32, tag=t, bufs=b,
                          name=f"ops{last-1}_{i}")
                  for i, (t, b) in enumerate((("st", 2), ("st", 2),
                                              ("out", 1), ("sums", 1)))]
            ch2 = [(cc_out[last - 1], dc * 128, dc) for dc in range(N_DC)]
            p4_run(last - 1, ch2, o2, 0, N_DC)
            p4_evict(last - 1, o2)
            p4_run(last, ch3[16:], o3, 16, N_DC)
            p4_evict(last, o3)

    nc.finalize()
    return nc


_MODULE_CACHE: dict = {}


def _get_module(causal: bool):
    if causal not in _MODULE_CACHE:
        _MODULE_CACHE[causal] = _build_module(causal)
    return _MODULE_CACHE[causal]


def _rope_perm() -> np.ndarray:
    """Per-head permutation: interleaved pairs (2j, 2j+1) -> (j, j+64)."""
    p = np.empty(HD, dtype=np.int64)
    p[0:64] = np.arange(0, HD, 2)
    p[64:128] = np.arange(1, HD, 2)
    full = np.concatenate([h * HD + p for h in range(HPC)])
    return full


def _canonical_causal_mask() -> np.ndarray:
    m = np.where(np.tril(np.ones((S, S), dtype=bool)), np.float32(0.0),
                 np.float32(NEG))
    return m.astype(np.float32)


def _numpy_fallback(x, freqs_cos, freqs_sin, mask, input_indexes, cache_k,
                    cache_v, wq, wk, wv, wo):
    """Exact reference reimplementation (host, fp32). Only used for inputs
    the device kernel does not model (non-arange cache indexes)."""
    B = x.shape[0]
    xf = x.astype(np.float32)

    def rope(t):
        tr = t[..., 0::2]
        ti = t[..., 1::2]
        c = freqs_cos[None, :, None, :]
        s = freqs_sin[None, :, None, :]
        outr = tr * c - ti * s
        outi = tr * s + ti * c
        return np.stack([outr, outi], axis=-1).reshape(t.shape)

    xq = (xf @ wq.T).reshape(B, S, N_HEADS, HD)
    xk = (xf @ wk.T).reshape(B, S, N_HEADS, HD)
    xv = (xf @ wv.T).reshape(B, S, N_HEADS, HD)
    xq = rope(xq)
    xk = rope(xk)
    keys = np.array(cache_k)
    vals = np.array(cache_v)
    keys[:, input_indexes] = xk
    vals[:, input_indexes] = xv
    scores = np.einsum("bqhd,bkhd->bhqk", xq, keys) / np.sqrt(HD)
    scores = scores + mask
    scores = scores - scores.max(axis=-1, keepdims=True)
    e = np.exp(scores)
    probs = e / e.sum(axis=-1, keepdims=True)
    out = np.einsum("bhqk,bkhd->bqhd", probs, vals)
    return (out.reshape(B, S, N_HEADS * HD) @ wo.T).astype(np.float32)


def _bf16(a: np.ndarray):
    import ml_dtypes
    return np.ascontiguousarray(a.astype(ml_dtypes.bfloat16))


def _prepare_in_maps(x, freqs_cos, freqs_sin, mask, wq, wk, wv, wo, causal):
    x2 = np.ascontiguousarray(x.reshape(S, D), dtype=np.float32)
    xTf = x2.T  # [D, S]
    xT = _bf16(np.concatenate(
        [xTf[:, sc * 512:(sc + 1) * 512] for sc in range(N_SC)], axis=0))

    cosb = np.empty((128, S), dtype=np.float32)
    sinb = np.empty((128, S), dtype=np.float32)
    fc = np.asarray(freqs_cos, dtype=np.float32).T  # [64, S]
    fs = np.asarray(freqs_sin, dtype=np.float32).T
    cosb[0:64] = fc
    cosb[64:128] = fc
    sinb[0:64] = fs
    sinb[64:128] = fs

    perm = _rope_perm()

    if causal:
        kl = np.arange(128, dtype=np.int64)[:, None]
        ql = np.arange(512, dtype=np.int64)[None, :]
        bmask = np.concatenate(
            [(kl <= ql - 128 * j).astype(np.float32) for j in range(4)], axis=1)
        bmask = _bf16(bmask)
    else:
        maskT = np.ascontiguousarray(
            (np.asarray(mask, dtype=np.float32)[0, 0].T) / np.float32(SCALE))

    in_maps = []
    for c in range(N_CORES):
        r0 = c * FC
        wq_c = np.asarray(wq[r0:r0 + FC], dtype=np.float32)[perm]
        wk_c = np.asarray(wk[r0:r0 + FC], dtype=np.float32)[perm]
        wqk_c = np.concatenate([wq_c, wk_c], axis=0)      # [1024, D]
        wv_c = np.asarray(wv[r0:r0 + FC], dtype=np.float32)
        wo_c = np.asarray(wo[r0:r0 + FC], dtype=np.float32)
        m = {
            "xT": xT,
            "wqk_t": _bf16(wqk_c.T),
            "wv_t": _bf16(wv_c.T),
            "wo_t": _bf16(wo_c.T),
            "cosb": cosb,
            "sinb": sinb,
        }
        if causal:
            m["bmask"] = bmask
        else:
            m["maskT"] = maskT
        in_maps.append(m)
    return in_maps


def _run(inputs: dict, trace: bool = False):
    x = np.asarray(inputs["x"])
    freqs_cos = np.asarray(inputs["freqs_cos"])
    freqs_sin = np.asarray(inputs["freqs_sin"])
    mask = np.asarray(inputs["mask"], dtype=np.float32)
    input_indexes = np.asarray(inputs["input_indexes"])
    wq = np.asarray(inputs["wq"])
    wk = np.asarray(inputs["wk"])
    wv = np.asarray(inputs["wv"])
    wo = np.asarray(inputs["wo"])

    if not np.array_equal(input_indexes.astype(np.int64), np.arange(S)):
        out = _numpy_fallback(x, freqs_cos, freqs_sin, mask, input_indexes,
                              inputs["cache_k"], inputs["cache_v"], wq, wk, wv, wo)
        return out, None

    causal = np.array_equal(mask[0, 0], _canonical_causal_mask())
    nc = _get_module(causal)
    in_maps = _prepare_in_maps(x, freqs_cos, freqs_sin, mask, wq, wk, wv, wo,
                               causal)
    res = run_bass_kernel_spmd(nc, in_maps, core_ids=list(range(N_CORES)),
                               trace=trace)
    out = np.concatenate([res.results[c]["out"] for c in range(N_CORES)],
                         axis=1)
    return out.reshape(1, S, D).astype(np.float32), res


def kernel(**inputs) -> np.ndarray:
    out, _ = _run(inputs, trace=False)
    return out
